# revision 16
# baseline (speedup 1.0000x reference)
"""CRF log-likelihood (sum over batch) on 8 Trainium2 NeuronCores.

Math (per batch element b):
    llh[b] = score(gold path) - logZ  (forward algorithm)
The forward recurrence runs on-device in the exp domain:
    u_0     = exp(start + em_0 - d)
    u_{t+1} = (u_t @ E) * exp(em_{t+1} - d),   E = exp(transitions)
    logZ    = log(sum_j u_{S-1}[j] * exp(end_j)) + S*d
where d is a constant per-step log-growth preconditioner (estimated on
host from 2 batch columns) that keeps u inside fp32/bf16 range, making
per-step renormalization (a partition-axis reduction) unnecessary.

Device mapping (per core, batch 64 = 2 groups of 32):
    partitions p = gi*64 + j  (gi in {0,1} batch half, j = tag)
    state u: [128, 32] bf16; per step one matmul with a block-diagonal
    stationary E+E [128,128] (q = u @ E for both groups at once), then one
    VectorE tensor_mul with the precomputed g = exp(em - d) slice.
    g is produced on-device by ScalarE Exp over DMA-streamed emissions.

The gold-path score only needs its batch SUM (output is sum over b), so
it reduces to global sums computed on-device in the chain's idle gaps:
one-hot tag masks are built with a K=2 broadcast matmul (group-selector
weights x tag rows) + a per-partition is_equal against an iota vector;
then fused scalar_tensor_tensor ops with accum_out accumulate
  sum em[t, tag, b]            (emission score)
  sum trans[tag_{t-1}, tag_t]  (via w = (T+T blockdiag) @ oh_{t-1}, . oh_t)
  sum start[tag_0], sum end[tag_last]
into per-partition accumulator columns, DMA'd out and summed on host.
"""

import numpy as np
import ml_dtypes

import concourse.bacc as bacc
import concourse.mybir as mybir
import concourse.tile as tile
from concourse.bass_utils import run_bass_kernel_spmd

S, B, T = 1024, 512, 64
NCORES = 8
BPC = B // NCORES          # 64 batch elements per core
GB = BPC // 2              # 32 per partition-group
CHUNK = 64                 # time steps per DMA/exp chunk
NCHUNK = S // CHUNK
QSTEP = 8                  # time steps per numerator quarter
QW = QSTEP * GB            # 256 columns
NQ = S // QSTEP            # 128 quarters
NACC = 2 * NQ + 2          # acc columns: em/trans per quarter + start/end

BF16 = ml_dtypes.bfloat16
F32 = mybir.dt.float32
BF = mybir.dt.bfloat16

_CACHE = {}


def build_nc(loop_reps=1, numerator=True):
    nc = bacc.Bacc("TRN2", target_bir_lowering=False, debug=False,
                   num_devices=NCORES)
    em = nc.dram_tensor("em", [128, S * GB], F32, kind="ExternalInput").ap()
    u0 = nc.dram_tensor("u0", [128, GB], BF, kind="ExternalInput").ap()
    eblk = nc.dram_tensor("eblk", [128, 128], BF, kind="ExternalInput").ap()
    negd = nc.dram_tensor("negd", [128, 1], F32, kind="ExternalInput").ap()
    uT = nc.dram_tensor("uT", [128, GB], BF, kind="ExternalOutput").ap()
    if numerator:
        tg = nc.dram_tensor("tg", [2, S * GB], BF, kind="ExternalInput").ap()
        tblk = nc.dram_tensor("tblk", [128, 128], BF,
                              kind="ExternalInput").ap()
        gsel = nc.dram_tensor("gsel", [2, 128], BF, kind="ExternalInput").ap()
        iot = nc.dram_tensor("iot", [128, 1], F32, kind="ExternalInput").ap()
        stv = nc.dram_tensor("stv", [128, 1], F32, kind="ExternalInput").ap()
        env = nc.dram_tensor("env", [128, 1], F32, kind="ExternalInput").ap()
        acc = nc.dram_tensor("acc", [128, NACC], F32,
                             kind="ExternalOutput").ap()

    with tile.TileContext(nc) as tc:
        with (
            tc.tile_pool(name="const", bufs=1) as constp,
            tc.tile_pool(name="g", bufs=NCHUNK) as gp,
            tc.tile_pool(name="stage", bufs=4) as stp,
            tc.tile_pool(name="u", bufs=1) as up,
            tc.tile_pool(name="q", bufs=4, space="PSUM") as qp,
            tc.tile_pool(name="bc", bufs=2, space="PSUM") as bcp,
            tc.tile_pool(name="w", bufs=2, space="PSUM") as wp,
            tc.tile_pool(name="oh", bufs=3) as ohp,
            tc.tile_pool(name="scr", bufs=3) as scp,
            tc.tile_pool(name="tgt", bufs=3) as tgp,
        ):
            def body(_iv=None):
                eb = constp.tile([128, 128], BF)
                nc.sync.dma_start(eb[:], eblk)
                nd = constp.tile([128, 1], F32)
                nc.sync.dma_start(nd[:], negd)

                # u arena: one slice per step, never recycled (avoids WAR
                # slot-recycle self-waits -> per-step EventSemaphore).
                ua = up.tile([128, S * GB], BF)
                nc.sync.dma_start(ua[:, 0:GB], u0)

                if numerator:
                    tb = constp.tile([128, 128], BF)
                    nc.sync.dma_start(tb[:], tblk)
                    gs = constp.tile([2, 128], BF)
                    nc.sync.dma_start(gs[:], gsel)
                    io_t = constp.tile([128, 1], F32)
                    nc.sync.dma_start(io_t[:], iot)
                    st_t = constp.tile([128, 1], F32)
                    nc.sync.dma_start(st_t[:], stv)
                    en_t = constp.tile([128, 1], F32)
                    nc.sync.dma_start(en_t[:], env)
                    acc_t = constp.tile([128, NACC], F32)

                gts, stgs, tgts = [], [], []
                for c in range(NCHUNK):
                    stg = stp.tile([128, CHUNK * GB], F32)
                    nc.sync.dma_start(
                        stg[:], em[:, c * CHUNK * GB:(c + 1) * CHUNK * GB])
                    stgs.append(stg)
                    gt = gp.tile([128, CHUNK * GB], BF)
                    nc.scalar.activation(gt[:], stg[:],
                                         mybir.ActivationFunctionType.Exp,
                                         bias=nd[:], scale=1.0)
                    gts.append(gt)
                    if numerator:
                        tgt = tgp.tile([2, CHUNK * GB], BF)
                        nc.sync.dma_start(
                            tgt[:],
                            tg[:, c * CHUNK * GB:(c + 1) * CHUNK * GB])
                        tgts.append(tgt)

                # numerator per-quarter state
                ohs = [None] * NQ     # one-hot tiles
                bcs = [None] * NQ
                ws = [None] * NQ
                mul = mybir.AluOpType.mult

                def num_op(t):
                    """Emit one numerator op at chain-step slot t (keeps at
                    most one extra DVE op between consecutive chain TTs)."""
                    q, ph = divmod(t - 1, QSTEP)
                    if q >= NQ:
                        return
                    c, qo = divmod(q, CHUNK // QSTEP)  # chunk, quarter-in-chunk
                    if ph == 0:      # PE: broadcast tags -> bc psum
                        bc = bcp.tile([128, QW], F32)
                        nc.tensor.matmul(
                            bc[:], lhsT=gs[:],
                            rhs=tgts[c][:, qo * QW:(qo + 1) * QW],
                            start=True, stop=True)
                        bcs[q] = bc
                    elif ph == 1:    # DVE: one-hot = (bc == iota)
                        oh = ohp.tile([128, QW], BF)
                        nc.vector.tensor_scalar(
                            oh[:], bcs[q][:], io_t[:], None,
                            mybir.AluOpType.is_equal)
                        ohs[q] = oh
                    elif ph == 2:    # PE: w = (T+T) @ oh_{t-1}
                        w = wp.tile([128, QW], F32)
                        if q > 0:
                            nc.tensor.matmul(
                                w[:, 0:GB], lhsT=tb[:],
                                rhs=ohs[q - 1][:, QW - GB:QW],
                                start=True, stop=True)
                        nc.tensor.matmul(
                            w[:, GB:QW], lhsT=tb[:],
                            rhs=ohs[q][:, 0:QW - GB],
                            start=True, stop=True)
                        ws[q] = w
                    elif ph == 3:    # DVE: emission score accum
                        scr = scp.tile([128, QW], F32)
                        nc.vector.scalar_tensor_tensor(
                            scr[:], stgs[c][:, qo * QW:(qo + 1) * QW],
                            1.0, ohs[q][:], mul, mul,
                            accum_out=acc_t[:, 2 * q:2 * q + 1])
                    elif ph == 4:    # DVE: transition score accum
                        scr = scp.tile([128, QW], F32)
                        if q > 0:
                            nc.vector.scalar_tensor_tensor(
                                scr[:], ws[q][:], 1.0, ohs[q][:], mul, mul,
                                accum_out=acc_t[:, 2 * q + 1:2 * q + 2])
                        else:
                            nc.vector.scalar_tensor_tensor(
                                scr[:, GB:QW], ws[q][:, GB:QW], 1.0,
                                ohs[q][:, GB:QW], mul, mul,
                                accum_out=acc_t[:, 1:2])
                    elif ph == 5 and q == 0:   # start-transition score
                        scr = scp.tile([128, QW], F32)
                        nc.vector.scalar_tensor_tensor(
                            scr[:, 0:GB], ohs[0][:, 0:GB], st_t[:],
                            ohs[0][:, 0:GB], mul, mul,
                            accum_out=acc_t[:, 2 * NQ:2 * NQ + 1])
                    elif ph == 5 and q == NQ - 1:  # end-transition score
                        scr = scp.tile([128, QW], F32)
                        nc.vector.scalar_tensor_tensor(
                            scr[:, 0:GB], ohs[q][:, QW - GB:QW], en_t[:],
                            ohs[q][:, QW - GB:QW], mul, mul,
                            accum_out=acc_t[:, 2 * NQ + 1:2 * NQ + 2])

                for t in range(1, S):
                    q = qp.tile([128, GB], F32)
                    nc.tensor.matmul(q[:], lhsT=eb[:],
                                     rhs=ua[:, (t - 1) * GB:t * GB],
                                     start=True, stop=True)
                    g_ap = gts[t // CHUNK][:, (t % CHUNK) * GB:
                                           ((t % CHUNK) + 1) * GB]
                    nc.vector.tensor_mul(ua[:, t * GB:(t + 1) * GB], q[:],
                                         g_ap)
                    if numerator:
                        num_op(t)
                # last quarter's phase-5 slot (t would be S..): emit directly
                if numerator:
                    num_op(S)  # no-op guard (q==NQ) keeps indexing safe

                nc.sync.dma_start(uT, ua[:, (S - 1) * GB:S * GB])
                if numerator:
                    nc.sync.dma_start(acc, acc_t[:])

            for _ in range(loop_reps):
                body()
    nc.compile()
    return nc


def _get_nc():
    if "nc" not in _CACHE:
        _CACHE["nc"] = build_nc()
    return _CACHE["nc"]


def _estimate_d(em, st, tr):
    """Per-step log-growth of the forward recurrence, from 2 batch columns."""
    sub = em[:, :2, :].astype(np.float64)
    Ed = np.exp(tr.astype(np.float64))
    alpha = st.astype(np.float64)[None, :] + sub[0]
    for t in range(1, S):
        m = alpha.max(axis=1, keepdims=True)
        alpha = m + np.log(np.exp(alpha - m) @ Ed) + sub[t]
    return float(alpha.max(axis=1).mean() / S)


def _host_inputs(em, st, tr, d, tags=None, en=None):
    """Per-core input maps for the device program."""
    E = np.exp(tr, dtype=np.float64)
    eblk = np.zeros((128, 128), np.float64)
    eblk[0:64, 0:64] = E
    eblk[64:128, 64:128] = E
    eblk = eblk.astype(BF16)
    negd = np.full((128, 1), -d, np.float32)
    numerator = tags is not None
    if numerator:
        tblk = np.zeros((128, 128), np.float32)
        tblk[0:64, 0:64] = tr
        tblk[64:128, 64:128] = tr
        tblk = tblk.astype(BF16)
        gsel = np.zeros((2, 128), np.float32)
        gsel[0, 0:64] = 1.0
        gsel[1, 64:128] = 1.0
        gsel = gsel.astype(BF16)
        iot = np.tile(np.arange(64, dtype=np.float32), 2)[:, None].copy()
        stv = np.tile(st, 2)[:, None].astype(np.float32).copy()
        env = np.tile(en, 2)[:, None].astype(np.float32).copy()
    in_maps = []
    for c in range(NCORES):
        x = em[:, BPC * c:BPC * (c + 1), :]                # (S, 64, T)
        xr = np.ascontiguousarray(
            x.reshape(S, 2, GB, T).transpose(1, 3, 0, 2)   # (gi, j, t, b')
        ).reshape(128, S * GB).astype(np.float32)
        u0 = np.exp(st[None, :].astype(np.float64)
                    + x[0].astype(np.float64) - d)          # (64b, T)
        u0 = np.ascontiguousarray(
            u0.reshape(2, GB, T).transpose(0, 2, 1)         # (gi, j, b')
        ).reshape(128, GB).astype(BF16)
        m = {"em": xr, "u0": u0, "eblk": eblk, "negd": negd}
        if numerator:
            tc_ = tags[:, BPC * c:BPC * (c + 1)]           # (S, 64)
            tgr = np.ascontiguousarray(
                tc_.reshape(S, 2, GB).transpose(1, 0, 2)   # (gi, t, b')
            ).reshape(2, S * GB).astype(BF16)
            m.update({"tg": tgr, "tblk": tblk, "gsel": gsel, "iot": iot,
                      "stv": stv, "env": env})
        in_maps.append(m)
    return in_maps


def _numerator(em, tags, mask_f, st, en, tr):
    tags = tags.astype(np.int64)
    emit = np.take_along_axis(em, tags[:, :, None], axis=2)[:, :, 0]
    emit = emit.astype(np.float64)
    score = st.astype(np.float64)[tags[0]] + emit[0]
    trans = tr[tags[:-1], tags[1:]].astype(np.float64)
    score = score + ((trans + emit[1:])
                     * mask_f[1:].astype(np.float64)).sum(0)
    seq_ends = mask_f.astype(np.int64).sum(0) - 1
    last_tags = tags[seq_ends, np.arange(tags.shape[1])]
    return score + en.astype(np.float64)[last_tags]


def _host_reference(em, tags, mask_f, st, en, tr):
    """Exact fp64 fallback (used only if mask is not all ones)."""
    Ed = np.exp(tr.astype(np.float64))
    alpha = st.astype(np.float64)[None, :] + em[0].astype(np.float64)
    for t in range(1, S):
        m = alpha.max(axis=1, keepdims=True)
        nxt = m + np.log(np.exp(alpha - m) @ Ed) + em[t].astype(np.float64)
        alpha = np.where(mask_f[t][:, None] > 0, nxt, alpha)
    m = alpha.max(axis=1)
    den = m + np.log(
        np.exp(alpha - m[:, None] + en.astype(np.float64)[None, :]).sum(1))
    num = _numerator(em, tags, mask_f, st, en, tr)
    return np.array((num - den).sum(), dtype=np.float32)


def kernel(emissions, tags, mask, start_transitions, end_transitions,
           transitions):
    em = np.asarray(emissions, np.float32)
    tags = np.asarray(tags)
    mask = np.asarray(mask)
    st = np.asarray(start_transitions, np.float32)
    en = np.asarray(end_transitions, np.float32)
    tr = np.asarray(transitions, np.float32)
    mask_f = (mask != 0).astype(np.float32)

    if not bool((mask != 0).all()):
        return _host_reference(em, tags, mask_f, st, en, tr)

    d = _estimate_d(em, st, tr)
    in_maps = _host_inputs(em, st, tr, d, tags=tags, en=en)
    nc = _get_nc()
    results = run_bass_kernel_spmd(nc, in_maps,
                                   core_ids=list(range(NCORES))).results

    en64 = np.exp(en.astype(np.float64))
    den = np.empty(B, np.float64)
    num_total = 0.0
    for c in range(NCORES):
        uT = np.asarray(results[c]["uT"]).astype(np.float64)  # [128, GB]
        u = uT.reshape(2, T, GB)                              # (gi, j, b')
        r = np.einsum("gjb,j->gb", u, en64)                   # (2, GB)
        den[BPC * c:BPC * (c + 1)] = (np.log(r) + d * S).reshape(BPC)
        num_total += float(np.asarray(results[c]["acc"])
                           .astype(np.float64).sum())

    return np.array(num_total - den.sum(), dtype=np.float32)
